# revision 17
# baseline (speedup 1.0000x reference)
"""Trainium2 Bass kernel for AbsSeq2SeqLSTM (B=256, S=T=128, H=512).

Architecture (8 NeuronCores, SPMD, one shared program):
  - Data-parallel over batch. Core i outputs batch rows [32*i, 32*(i+1)).
  - Phases per core:
      P0: gather input/output embeddings (indirect DMA)
      P1: zx GEMMs  (x @ Wx_e + b_e, y_emb @ Wx_d + b_d) in fp32r
      P2: encoder LSTM scan(s): PE recurrent matmul + zx injection via
          identity matmul, gates on ACT, state update on DVE,
          h re-transposed on PE each step
      P3: handoff (h, c) to decoder (local add or pairwise ReduceScatter)
      P4: decoder scan; h_T staged to DRAM for the projection
      P5: output projection GEMM (HdT.T @ W_out + b_out)
      P6: softmax (exp with fused row-sum) + scatter-store to output
  - MODE "quarter": core 2j runs enc_f on batch quarter j, core 2j+1 runs
    enc_b on the same quarter; ReduceScatter(add) over pairs hands each
    core the summed (h, c) for its decode half.
  - MODE "eighth": each core runs both encoder cells on its batch eighth;
    no collectives.
"""

import os
from contextlib import ExitStack

import numpy as np

import concourse.bacc as bacc
import concourse.bass as bass
import concourse.mybir as mybir
import concourse.tile as tile
from concourse.bass_utils import run_bass_kernel_spmd
from concourse.kernels.tile_matmul import matmul_tile_kernel
from concourse.masks import make_identity
from concourse.tile import TileContext

_IDENT_F32 = None  # f32 [128,128] identity AP, set once per build


def _make_identity_any(nc, identity, nomemset=False):
    """make_identity for fp32r tiles: the BIR verifier requires every writer
    of an fp32r matmul operand to produce rounded fp32r, so copy (with
    rounding) from a prebuilt f32 identity."""
    ap = identity if isinstance(identity, bass.AP) else identity[:]
    if ap.dtype == mybir.dt.float32r:
        assert _IDENT_F32 is not None
        nc.vector.tensor_copy(out=ap, in_=_IDENT_F32[0:ap.shape[0], 0:ap.shape[1]])
    else:
        make_identity(nc, ap, nomemset=nomemset)


import concourse.kernels.tile_matmul as _tm

_tm.make_identity = _make_identity_any

AF = mybir.ActivationFunctionType
ALU = mybir.AluOpType
F32 = mybir.dt.float32
F32R = mybir.dt.float32r
I32 = mybir.dt.int32

B_TOT, S_FULL, T_FULL, H = 256, 128, 128, 512
G4 = 4 * H  # 2048
IN_VOCAB, OUT_VOCAB = 32000, 1024
NCORES = 8
P = 128

MODE = os.environ.get("KERNEL_MODE", "quarter")  # "quarter" | "eighth"
S_STEPS = int(os.environ.get("KERNEL_STEPS", str(S_FULL)))  # test hook


def _r(ap):
    return ap.bitcast(F32R)


class LSTMCell:
    """On-chip state for one LSTM cell: resident Wh, h (transposed), c."""

    def __init__(self, nc, tc, ctx, name, B, wh_dram):
        self.B = B
        pool = ctx.enter_context(tc.tile_pool(name=f"st_{name}", bufs=1))
        self.wh = pool.tile([P, 4, G4], F32R, tag=f"wh_{name}")
        for k in range(4):
            nc.sync.dma_start(out=self.wh[:, k, :], in_=wh_dram[k * P:(k + 1) * P, :])
        self.hT = pool.tile([P, 4, B], F32R, tag=f"hT_{name}")
        self.c = pool.tile([B, H], F32, tag=f"c_{name}")
        nc.vector.memset(self.c[:], 0.0)


def lstm_step(nc, pools, cell, zx_src, skip_htranspose=False, first_step=False):
    """One LSTM step. zx_src: SBUF tile [B, 4, 512] fp32 with the
    x-projection (+bias). Updates cell.hT / cell.c. Returns natural h."""
    B = cell.B
    zps, tps, gates, ident = pools["zps"], pools["tps"], pools["gates"], pools["ident"]

    g_sb = gates.tile([B, 4, 512], F32, tag="g_sb")
    for n in range(4):  # gate chunks: i, f, g, o
        zp = zps.tile([B, 512], F32, tag="zp")
        nc.tensor.matmul(zp[:], ident[:B, :B], zx_src[:, n, :],
                         start=True, stop=first_step)
        if not first_step:
            for k in range(4):
                nc.tensor.matmul(zp[:], cell.hT[:, k, :],
                                 cell.wh[:, k, n * 512:(n + 1) * 512],
                                 start=False, stop=(k == 3))
        func = AF.Tanh if n == 2 else AF.Sigmoid
        nc.scalar.activation(g_sb[:, n, :], zp[:], func)

    i_g, f_g, g_g, o_g = (g_sb[:, n, :] for n in range(4))
    t1 = gates.tile([B, H], F32, tag="t1")
    nc.vector.tensor_tensor(out=t1[:], in0=f_g, in1=cell.c[:], op=ALU.mult)
    t2 = gates.tile([B, H], F32, tag="t2")
    nc.vector.tensor_tensor(out=t2[:], in0=i_g, in1=g_g, op=ALU.mult)
    nc.vector.tensor_tensor(out=cell.c[:], in0=t1[:], in1=t2[:], op=ALU.add)
    tc_t = gates.tile([B, H], F32, tag="tc")
    nc.scalar.activation(tc_t[:], cell.c[:], AF.Tanh)
    h_sb = gates.tile([B, H], F32R, tag="h_sb")
    nc.vector.tensor_tensor(out=h_sb[:], in0=o_g, in1=tc_t[:], op=ALU.mult)
    if not skip_htranspose:
        transpose_into(nc, pools, h_sb, cell.hT, B)
    return h_sb


def transpose_into(nc, pools, h_nat, hT_dst, B):
    tps, ident = pools["tps"], pools["ident"]
    for k in range(4):
        tp = tps.tile([P, B], F32R, tag="tp")
        nc.tensor.transpose(tp[:], h_nat[:, k * P:(k + 1) * P], ident[:B, :B])
        nc.vector.tensor_copy(out=hT_dst[:, k, :], in_=tp[:])


def build_program(B_enc, n_enc_cells, B_dec, use_rs, n_steps):
    S = T = n_steps
    nc = bacc.Bacc()

    x_idx = nc.dram_tensor("x_idx", [S * B_enc, 1], I32, kind="ExternalInput")
    y_idx = nc.dram_tensor("y_idx", [T * B_dec, 1], I32, kind="ExternalInput")
    in_emb = nc.dram_tensor("in_emb", [IN_VOCAB, H], F32R, kind="ExternalInput")
    out_emb = nc.dram_tensor("out_emb", [OUT_VOCAB, OUT_VOCAB], F32R, kind="ExternalInput")
    wx_e = [nc.dram_tensor(f"Wx_e{i}", [H, G4], F32R, kind="ExternalInput") for i in range(n_enc_cells)]
    wh_e = [nc.dram_tensor(f"Wh_e{i}", [H, G4], F32R, kind="ExternalInput") for i in range(n_enc_cells)]
    b_e = [nc.dram_tensor(f"b_e{i}", [1, G4], F32, kind="ExternalInput") for i in range(n_enc_cells)]
    wx_d = nc.dram_tensor("Wx_d", [OUT_VOCAB, G4], F32R, kind="ExternalInput")
    wh_d = nc.dram_tensor("Wh_d", [H, G4], F32R, kind="ExternalInput")
    b_d = nc.dram_tensor("b_d", [1, G4], F32, kind="ExternalInput")
    w_out = nc.dram_tensor("W_out", [H, OUT_VOCAB], F32R, kind="ExternalInput")
    b_out = nc.dram_tensor("b_out", [1, OUT_VOCAB], F32, kind="ExternalInput")
    out = nc.dram_tensor("out", [B_dec, T, OUT_VOCAB], F32, kind="ExternalOutput")
    debug = os.environ.get("KERNEL_DEBUG", "0") == "1"
    dbg = {}

    x_stage = nc.dram_tensor("x_stage", [S * B_enc, H], F32R)
    y_stage = nc.dram_tensor("y_stage", [T * B_dec, OUT_VOCAB], F32R)
    zx_e = [nc.dram_tensor(f"zx_e{i}", [S * B_enc, G4], F32R) for i in range(n_enc_cells)]
    zx_d = nc.dram_tensor("zx_d", [T * B_dec, G4], F32R)
    hdt = nc.dram_tensor("hdt", [4 * P, T * B_dec], F32R)
    logits = nc.dram_tensor("logits", [T * B_dec, OUT_VOCAB], F32)
    if debug:
        dbg["x_stage"] = nc.dram_tensor("dbg_x", [S * B_enc, H], F32, kind="ExternalOutput")
        dbg["zx_e0"] = nc.dram_tensor("dbg_zx0", [S * B_enc, G4], F32, kind="ExternalOutput")
        dbg["zx_d"] = nc.dram_tensor("dbg_zxd", [T * B_dec, G4], F32, kind="ExternalOutput")
        dbg["hdt"] = nc.dram_tensor("dbg_hdt", [4 * P, T * B_dec], F32, kind="ExternalOutput")
        dbg["logits"] = nc.dram_tensor("dbg_logits", [T * B_dec, OUT_VOCAB], F32, kind="ExternalOutput")
        dbg["et"] = nc.dram_tensor("dbg_et", [P, OUT_VOCAB], F32, kind="ExternalOutput")
        dbg["rs"] = nc.dram_tensor("dbg_rs", [P, 1], F32, kind="ExternalOutput")
        dbg["rr"] = nc.dram_tensor("dbg_rr", [P, 1], F32, kind="ExternalOutput")
        dbg["ot"] = nc.dram_tensor("dbg_ot", [P, OUT_VOCAB], F32, kind="ExternalOutput")
        dbg["lt"] = nc.dram_tensor("dbg_lt", [P, OUT_VOCAB], F32, kind="ExternalOutput")
    if use_rs:
        rs_h_in = nc.dram_tensor("rs_h_in", [B_enc, H], F32R)
        rs_c_in = nc.dram_tensor("rs_c_in", [B_enc, H], F32)
        rs_h_out = nc.dram_tensor("rs_h_out", [B_dec, H], F32R)
        rs_c_out = nc.dram_tensor("rs_c_out", [B_dec, H], F32)
        rs_groups = [[2 * j, 2 * j + 1] for j in range(4)]

    with TileContext(nc) as tc, ExitStack() as octx:
        const = octx.enter_context(tc.tile_pool(name="const", bufs=1))
        ident_f32 = const.tile([P, P], F32)
        make_identity(nc, ident_f32[:])
        global _IDENT_F32
        _IDENT_F32 = ident_f32
        ident = const.tile([P, P], F32R)
        _make_identity_any(nc, ident[:])
        dstate = octx.enter_context(tc.tile_pool(name="dstate", bufs=1))
        h0 = dstate.tile([B_dec, H], F32R, tag="h0")
        c0 = dstate.tile([B_dec, H], F32, tag="c0")

        # ---------- P0: embedding gathers ----------
        def gather(table, idx_dram, n_rows, width, stage):
            with tc.tile_pool(name="gth", bufs=4) as gp, tc.tile_pool(name="gthi", bufs=4) as gip:
                for blk in range(n_rows // P):
                    it = gip.tile([P, 1], I32, tag="idx")
                    nc.sync.dma_start(out=it[:], in_=idx_dram[blk * P:(blk + 1) * P, :])
                    gt = gp.tile([P, width], F32R, tag="rows")
                    nc.gpsimd.indirect_dma_start(
                        out=gt[:], out_offset=None, in_=table[:],
                        in_offset=bass.IndirectOffsetOnAxis(ap=it[:, :1], axis=0),
                    )
                    nc.sync.dma_start(out=stage[blk * P:(blk + 1) * P, :], in_=gt[:])

        gather(in_emb, x_idx, S * B_enc, H, x_stage)
        gather(out_emb, y_idx, T * B_dec, OUT_VOCAB, y_stage)

        # ---------- P1: zx GEMMs ----------
        def bias_adder(bias_tile):
            def post(nc_, sbuf, md, _):
                # sbuf: [m_partitions, m_subtiles, n_cols]; bias varies on n only
                m = sbuf.shape[0]
                n0 = md.n_slice.start
                ncols = sbuf.shape[-1]
                bv = bias_tile[0:m, n0:n0 + ncols]
                if len(sbuf.shape) == 3:
                    bv = bv.unsqueeze(1).to_broadcast(tuple(sbuf.shape))
                nc_.vector.tensor_tensor(out=sbuf, in0=sbuf, in1=bv, op=ALU.add)
            return post

        def gemm(ctx, kxm, kxn, mxn, bias_dram, nbias, transpose_kxm=True):
            bt = ctx.enter_context(tc.tile_pool(name="bias", bufs=1)).tile([P, nbias], F32)
            nc.sync.dma_start(out=bt[:].unsqueeze(1), in_=bias_dram[0:1, :].partition_broadcast(P))
            matmul_tile_kernel(
                tc, kxm[:], kxn[:], mxn[:],
                transpose_kxm=transpose_kxm, force_tensor_transpose=transpose_kxm,
                post_mxn_tile_fn=bias_adder(bt),
            )

        with ExitStack() as ctx:
            for ci in range(n_enc_cells):
                gemm(ctx, x_stage, wx_e[ci], zx_e[ci], b_e[ci], G4)
            gemm(ctx, y_stage, wx_d, zx_d, b_d, G4)

        # ---------- P2: encoder scans ----------
        with ExitStack() as ctx:
            pools = dict(
                zps=ctx.enter_context(tc.tile_pool(name="zps", bufs=3, space="PSUM")),
                tps=ctx.enter_context(tc.tile_pool(name="tps", bufs=2, space="PSUM")),
                gates=ctx.enter_context(tc.tile_pool(name="gates", bufs=3)),
                ident=ident,
            )
            zxp = ctx.enter_context(tc.tile_pool(name="zxp", bufs=4))
            ecells = [LSTMCell(nc, tc, ctx, f"e{i}", B_enc, wh_e[i]) for i in range(n_enc_cells)]
            h_fin = []
            for t in range(S):
                last = t == S - 1
                for cell, zxd_ in zip(ecells, zx_e):
                    zt = zxp.tile([B_enc, 4, 512], F32R, tag="zt")
                    nc.sync.dma_start(
                        out=zt[:],
                        in_=zxd_[t * B_enc:(t + 1) * B_enc, :].rearrange("b (n f) -> b n f", n=4))
                    h = lstm_step(nc, pools, cell, zt, skip_htranspose=last,
                                  first_step=(t == 0))
                    if last:
                        h_fin.append(h)

            # ---------- P3: handoff ----------
            if use_rs:
                nc.sync.dma_start(out=rs_h_in[:], in_=h_fin[0][:])
                nc.sync.dma_start(out=rs_c_in[:], in_=ecells[0].c[:])
                nc.gpsimd.collective_compute(
                    "ReduceScatter", ALU.add, ins=[rs_h_in[:]], outs=[rs_h_out[:]],
                    replica_groups=rs_groups)
                nc.gpsimd.collective_compute(
                    "ReduceScatter", ALU.add, ins=[rs_c_in[:]], outs=[rs_c_out[:]],
                    replica_groups=rs_groups)
                nc.sync.dma_start(out=h0[:], in_=rs_h_out[:])
                nc.sync.dma_start(out=c0[:], in_=rs_c_out[:])
            else:
                nc.vector.tensor_tensor(out=h0[:], in0=h_fin[0][:], in1=h_fin[1][:], op=ALU.add)
                nc.vector.tensor_tensor(out=c0[:], in0=ecells[0].c[:], in1=ecells[1].c[:], op=ALU.add)

        # ---------- P4: decoder scan ----------
        with ExitStack() as ctx:
            pools = dict(
                zps=ctx.enter_context(tc.tile_pool(name="zpsd", bufs=3, space="PSUM")),
                tps=ctx.enter_context(tc.tile_pool(name="tpsd", bufs=2, space="PSUM")),
                gates=ctx.enter_context(tc.tile_pool(name="gatesd", bufs=3)),
                ident=ident,
            )
            zxp = ctx.enter_context(tc.tile_pool(name="zxpd", bufs=4))
            dcell = LSTMCell(nc, tc, ctx, "d", B_dec, wh_d)
            transpose_into(nc, pools, h0, dcell.hT, B_dec)
            nc.vector.tensor_copy(out=dcell.c[:], in_=c0[:])

            for t in range(T):
                zt = zxp.tile([B_dec, 4, 512], F32R, tag="zt")
                nc.sync.dma_start(
                    out=zt[:],
                    in_=zx_d[t * B_dec:(t + 1) * B_dec, :].rearrange("b (n f) -> b n f", n=4))
                lstm_step(nc, pools, dcell, zt)
                nc.sync.dma_start(
                    out=hdt[:, t * B_dec:(t + 1) * B_dec].rearrange("(k p) b -> p k b", p=P),
                    in_=dcell.hT[:],
                )

        # ---------- P5: output projection ----------
        with ExitStack() as ctx:
            bt = ctx.enter_context(tc.tile_pool(name="biaso", bufs=1)).tile([P, OUT_VOCAB], F32)
            nc.sync.dma_start(out=bt[:].unsqueeze(1), in_=b_out[0:1, :].partition_broadcast(P))
            matmul_tile_kernel(
                tc, hdt[:], w_out[:], logits[:],
                post_mxn_tile_fn=bias_adder(bt),
            )

        # ---------- P6: softmax + scatter ----------
        with ExitStack() as ctx:
            sp = ctx.enter_context(tc.tile_pool(name="smax", bufs=3))
            rp = ctx.enter_context(tc.tile_pool(name="smr", bufs=3))
            t_per = P // B_dec
            for blk in range(T * B_dec // P):
                lt = sp.tile([P, OUT_VOCAB], F32, tag="lt")
                nc.sync.dma_start(out=lt[:], in_=logits[blk * P:(blk + 1) * P, :])
                et = sp.tile([P, OUT_VOCAB], F32, tag="et")
                rs = rp.tile([P, 1], F32, tag="rs")
                nc.scalar.activation(et[:], lt[:], AF.Exp)
                nc.vector.reduce_sum(out=rs[:], in_=et[:], axis=mybir.AxisListType.X)
                rr = rp.tile([P, 1], F32, tag="rr")
                nc.vector.reciprocal(rr[:], rs[:])
                ot = sp.tile([P, OUT_VOCAB], F32, tag="ot")
                nc.vector.tensor_scalar(ot[:], et[:], rr[:], None, ALU.mult)
                if debug and blk == 0:
                    nc.sync.dma_start(out=dbg["lt"][:], in_=lt[:])
                    nc.sync.dma_start(out=dbg["et"][:], in_=et[:])
                    nc.sync.dma_start(out=dbg["rs"][:], in_=rs[:])
                    nc.sync.dma_start(out=dbg["rr"][:], in_=rr[:])
                    nc.sync.dma_start(out=dbg["ot"][:], in_=ot[:])
                t0 = blk * t_per
                for j in range(t_per):
                    nc.sync.dma_start(
                        out=out[:, t0 + j, :],
                        in_=ot[j * B_dec:(j + 1) * B_dec, :],
                    )

        if debug:
            nc.sync.dma_start(out=dbg["x_stage"][:], in_=x_stage[:].bitcast(F32))
            nc.sync.dma_start(out=dbg["zx_e0"][:], in_=zx_e[0][:].bitcast(F32))
            nc.sync.dma_start(out=dbg["zx_d"][:], in_=zx_d[:].bitcast(F32))
            nc.sync.dma_start(out=dbg["hdt"][:], in_=hdt[:].bitcast(F32))
            nc.sync.dma_start(out=dbg["logits"][:], in_=logits[:])

    nc.finalize()
    return nc


_PROG_CACHE = {}


def _get_program(key):
    if key not in _PROG_CACHE:
        _PROG_CACHE[key] = build_program(*key)
    return _PROG_CACHE[key]


def kernel(inputs, teacher_actions, max_length, in_emb, out_emb,
           Wx_f, Wh_f, b_f, Wx_b, Wh_b, b_b,
           Wx_d, Wh_d, b_d, W_out, b_out, _trace=False):
    inputs = np.asarray(inputs)
    teacher_actions = np.asarray(teacher_actions)
    n_steps = S_STEPS
    assert int(max_length) == T_FULL and inputs.shape == (B_TOT, S_FULL)
    f32 = lambda a: np.ascontiguousarray(np.asarray(a), dtype=np.float32)
    in_emb, out_emb = f32(in_emb), f32(out_emb)
    Wx_f, Wh_f, Wx_b, Wh_b = f32(Wx_f), f32(Wh_f), f32(Wx_b), f32(Wh_b)
    Wx_d, Wh_d, W_out = f32(Wx_d), f32(Wh_d), f32(W_out)
    b_f, b_b, b_d, b_out = (f32(b).reshape(1, -1) for b in (b_f, b_b, b_d, b_out))

    if MODE == "quarter":
        B_enc, n_cells, B_dec, use_rs = 64, 1, 32, True
    else:
        B_enc, n_cells, B_dec, use_rs = 32, 2, 32, False
    nc = _get_program((B_enc, n_cells, B_dec, use_rs, n_steps))

    in_maps = []
    for core in range(NCORES):
        bs_d = core * B_dec
        m = dict(in_emb=in_emb, out_emb=out_emb, Wx_d=Wx_d, Wh_d=Wh_d, b_d=b_d,
                 W_out=W_out, b_out=b_out)
        m["y_idx"] = np.ascontiguousarray(
            teacher_actions[:n_steps, bs_d:bs_d + B_dec].reshape(-1, 1).astype(np.int32))
        if use_rs:
            q = core // 2
            rows = np.arange(64 * q, 64 * (q + 1))
            wx, wh, b = (Wx_f, Wh_f, b_f) if core % 2 == 0 else (Wx_b, Wh_b, b_b)
            m["Wx_e0"], m["Wh_e0"], m["b_e0"] = wx, wh, b
        else:
            rows = np.arange(bs_d, bs_d + B_enc)
            m["Wx_e0"], m["Wh_e0"], m["b_e0"] = Wx_f, Wh_f, b_f
            m["Wx_e1"], m["Wh_e1"], m["b_e1"] = Wx_b, Wh_b, b_b
        m["x_idx"] = np.ascontiguousarray(
            inputs[rows, :n_steps].T.reshape(-1, 1).astype(np.int32))
        in_maps.append(m)

    res = run_bass_kernel_spmd(nc, in_maps, core_ids=list(range(NCORES)), trace=_trace)
    out = np.concatenate([res.results[i]["out"] for i in range(NCORES)], axis=0)
    if _trace:
        kernel.last_exec_time_ns = res.exec_time_ns
        kernel.last_results = res
    return out


# revision 19
# speedup vs baseline: 93.0064x; 93.0064x over previous
"""Trainium2 Bass kernel for AbsSeq2SeqLSTM (B=256, S=T=128, H=512).

Architecture (8 NeuronCores, SPMD, one shared program):
  - Data-parallel over batch. Core i outputs batch rows [32*i, 32*(i+1)).
  - Phases per core:
      P0: gather input/output embeddings (indirect DMA)
      P1: zx GEMMs  (x @ Wx_e + b_e, y_emb @ Wx_d + b_d) in fp32r
      P2: encoder LSTM scan(s): PE recurrent matmul + zx injection via
          identity matmul, gates on ACT, state update on DVE,
          h re-transposed on PE each step
      P3: handoff (h, c) to decoder (local add or pairwise ReduceScatter)
      P4: decoder scan; h_T staged to DRAM for the projection
      P5: output projection GEMM (HdT.T @ W_out + b_out)
      P6: softmax (exp with fused row-sum) + scatter-store to output
  - MODE "quarter": core 2j runs enc_f on batch quarter j, core 2j+1 runs
    enc_b on the same quarter; ReduceScatter(add) over pairs hands each
    core the summed (h, c) for its decode half.
  - MODE "eighth": each core runs both encoder cells on its batch eighth;
    no collectives.
"""

import os
from contextlib import ExitStack

import numpy as np

import concourse.bacc as bacc
import concourse.bass as bass
import concourse.mybir as mybir
import concourse.tile as tile
from concourse.bass_utils import run_bass_kernel_spmd
from concourse.kernels.tile_matmul import matmul_tile_kernel
from concourse.masks import make_identity
from concourse.tile import TileContext

_IDENT_F32 = None  # f32 [128,128] identity AP, set once per build


def _make_identity_any(nc, identity, nomemset=False):
    """make_identity for fp32r tiles: the BIR verifier requires every writer
    of an fp32r matmul operand to produce rounded fp32r, so copy (with
    rounding) from a prebuilt f32 identity."""
    ap = identity if isinstance(identity, bass.AP) else identity[:]
    if ap.dtype == mybir.dt.float32r:
        assert _IDENT_F32 is not None
        nc.vector.tensor_copy(out=ap, in_=_IDENT_F32[0:ap.shape[0], 0:ap.shape[1]])
    else:
        make_identity(nc, ap, nomemset=nomemset)


import concourse.kernels.tile_matmul as _tm

_tm.make_identity = _make_identity_any

AF = mybir.ActivationFunctionType
ALU = mybir.AluOpType
F32 = mybir.dt.float32
F32R = mybir.dt.float32r
I32 = mybir.dt.int32

B_TOT, S_FULL, T_FULL, H = 256, 128, 128, 512
G4 = 4 * H  # 2048
IN_VOCAB, OUT_VOCAB = 32000, 1024
NCORES = 8
P = 128

MODE = os.environ.get("KERNEL_MODE", "quarter")  # "quarter" | "eighth"
S_STEPS = int(os.environ.get("KERNEL_STEPS", str(S_FULL)))  # test hook


def _r(ap):
    return ap.bitcast(F32R)


class LSTMCell:
    """On-chip state for one LSTM cell: resident Wh, h (transposed), c."""

    def __init__(self, nc, tc, ctx, name, B, wh_dram):
        self.B = B
        pool = ctx.enter_context(tc.tile_pool(name=f"st_{name}", bufs=1))
        self.wh = pool.tile([P, 4, G4], F32R, tag=f"wh_{name}")
        for k in range(4):
            nc.sync.dma_start(out=self.wh[:, k, :], in_=wh_dram[k * P:(k + 1) * P, :])
        self.hT = pool.tile([P, 4, B], F32R, tag=f"hT_{name}")
        self.c = pool.tile([B, H], F32, tag=f"c_{name}")
        nc.vector.memset(self.c[:], 0.0)


def lstm_step(nc, pools, cell, zx_src, skip_htranspose=False, first_step=False,
              hT_read=None, hT_write=None):
    """One LSTM step. zx_src: SBUF tile [B, 4, 512] fp32 with the
    x-projection (+bias). Updates h (transposed) and cell.c. Returns natural h."""
    B = cell.B
    zps, tps, gates, ident = pools["zps"], pools["tps"], pools["gates"], pools["ident"]
    if hT_read is None:
        hT_read = cell.hT[:]
    if hT_write is None:
        hT_write = cell.hT[:]

    g_sb = gates.tile([B, 4, 512], F32, tag="g_sb")
    for n in range(4):  # gate chunks: i, f, g, o
        zp = zps.tile([B, 512], F32, tag="zp")
        nc.tensor.matmul(zp[:], ident[:B, :B], zx_src[:, n, :],
                         start=True, stop=first_step)
        if not first_step:
            for k in range(4):
                nc.tensor.matmul(zp[:], hT_read[:, k, :],
                                 cell.wh[:, k, n * 512:(n + 1) * 512],
                                 start=False, stop=(k == 3))
        func = AF.Tanh if n == 2 else AF.Sigmoid
        nc.scalar.activation(g_sb[:, n, :], zp[:], func)

    i_g, f_g, g_g, o_g = (g_sb[:, n, :] for n in range(4))
    t1 = gates.tile([B, H], F32, tag="t1")
    nc.vector.tensor_tensor(out=t1[:], in0=f_g, in1=cell.c[:], op=ALU.mult)
    t2 = gates.tile([B, H], F32, tag="t2")
    nc.vector.tensor_tensor(out=t2[:], in0=i_g, in1=g_g, op=ALU.mult)
    nc.vector.tensor_tensor(out=cell.c[:], in0=t1[:], in1=t2[:], op=ALU.add)
    tc_t = gates.tile([B, H], F32, tag="tc")
    nc.scalar.activation(tc_t[:], cell.c[:], AF.Tanh)
    h_sb = gates.tile([B, H], F32R, tag="h_sb")
    nc.vector.tensor_tensor(out=h_sb[:], in0=o_g, in1=tc_t[:], op=ALU.mult)
    if not skip_htranspose:
        transpose_into(nc, pools, h_sb, hT_write, B)
    return h_sb


def transpose_into(nc, pools, h_nat, hT_dst, B):
    tps, ident = pools["tps"], pools["ident"]
    for k in range(4):
        tp = tps.tile([P, B], F32R, tag="tp")
        nc.tensor.transpose(tp[:], h_nat[:, k * P:(k + 1) * P], ident[:B, :B])
        nc.vector.tensor_copy(out=hT_dst[:, k, :], in_=tp[:])


SM_GROUP = 16  # decoder steps per softmax flush (ACT table-switch batching)


def build_program(B_enc, n_enc_cells, B_dec, use_rs, n_steps):
    S = T = n_steps
    nc = bacc.Bacc()

    x_idx = nc.dram_tensor("x_idx", [S * B_enc, 1], I32, kind="ExternalInput")
    y_idx = nc.dram_tensor("y_idx", [T * B_dec, 1], I32, kind="ExternalInput")
    in_emb = nc.dram_tensor("in_emb", [IN_VOCAB, H], F32R, kind="ExternalInput")
    out_emb = nc.dram_tensor("out_emb", [OUT_VOCAB, OUT_VOCAB], F32R, kind="ExternalInput")
    wx_e = [nc.dram_tensor(f"Wx_e{i}", [H, G4], F32R, kind="ExternalInput") for i in range(n_enc_cells)]
    wh_e = [nc.dram_tensor(f"Wh_e{i}", [H, G4], F32R, kind="ExternalInput") for i in range(n_enc_cells)]
    b_e = [nc.dram_tensor(f"b_e{i}", [1, G4], F32, kind="ExternalInput") for i in range(n_enc_cells)]
    wx_d = nc.dram_tensor("Wx_d", [OUT_VOCAB, G4], F32R, kind="ExternalInput")
    wh_d = nc.dram_tensor("Wh_d", [H, G4], F32R, kind="ExternalInput")
    b_d = nc.dram_tensor("b_d", [1, G4], F32, kind="ExternalInput")
    w_out = nc.dram_tensor("W_out", [H, OUT_VOCAB], F32R, kind="ExternalInput")
    b_out = nc.dram_tensor("b_out", [1, OUT_VOCAB], F32, kind="ExternalInput")
    out = nc.dram_tensor("out", [B_dec, T, OUT_VOCAB], F32, kind="ExternalOutput")
    debug = os.environ.get("KERNEL_DEBUG", "0") == "1"
    dbg = {}

    x_stage = nc.dram_tensor("x_stage", [S * B_enc, H], F32R)
    zx_e = [nc.dram_tensor(f"zx_e{i}", [S * B_enc, G4], F32R) for i in range(n_enc_cells)]
    g_tab = nc.dram_tensor("g_tab", [OUT_VOCAB, G4], F32R)
    if debug:
        dbg["x_stage"] = nc.dram_tensor("dbg_x", [S * B_enc, H], F32, kind="ExternalOutput")
        dbg["zx_e0"] = nc.dram_tensor("dbg_zx0", [S * B_enc, G4], F32, kind="ExternalOutput")
        dbg["g_tab"] = nc.dram_tensor("dbg_g", [OUT_VOCAB, G4], F32, kind="ExternalOutput")
    if use_rs:
        rs_h_in = nc.dram_tensor("rs_h_in", [B_enc, H], F32R)
        rs_c_in = nc.dram_tensor("rs_c_in", [B_enc, H], F32)
        rs_h_out = nc.dram_tensor("rs_h_out", [B_dec, H], F32R)
        rs_c_out = nc.dram_tensor("rs_c_out", [B_dec, H], F32)
        rs_groups = [[2 * j, 2 * j + 1] for j in range(4)]

    with TileContext(nc) as tc, ExitStack() as octx:
        const = octx.enter_context(tc.tile_pool(name="const", bufs=1))
        ident_f32 = const.tile([P, P], F32)
        make_identity(nc, ident_f32[:])
        global _IDENT_F32
        _IDENT_F32 = ident_f32
        ident = const.tile([P, P], F32R)
        _make_identity_any(nc, ident[:])
        dstate = octx.enter_context(tc.tile_pool(name="dstate", bufs=1))
        h0 = dstate.tile([B_dec, H], F32R, tag="h0")
        c0 = dstate.tile([B_dec, H], F32, tag="c0")

        # ---------- P0: embedding gathers ----------
        def gather(table, idx_dram, n_rows, width, stage):
            with tc.tile_pool(name="gth", bufs=4) as gp, tc.tile_pool(name="gthi", bufs=4) as gip:
                for blk in range(n_rows // P):
                    it = gip.tile([P, 1], I32, tag="idx")
                    nc.sync.dma_start(out=it[:], in_=idx_dram[blk * P:(blk + 1) * P, :])
                    gt = gp.tile([P, width], F32R, tag="rows")
                    nc.gpsimd.indirect_dma_start(
                        out=gt[:], out_offset=None, in_=table[:],
                        in_offset=bass.IndirectOffsetOnAxis(ap=it[:, :1], axis=0),
                    )
                    nc.sync.dma_start(out=stage[blk * P:(blk + 1) * P, :], in_=gt[:])

        gather(in_emb, x_idx, S * B_enc, H, x_stage)

        # ---------- P1: zx GEMMs ----------
        def bias_adder(bias_tile):
            def post(nc_, sbuf, md, _):
                # sbuf: [m_partitions, m_subtiles, n_cols]; bias varies on n only
                m = sbuf.shape[0]
                n0 = md.n_slice.start
                ncols = sbuf.shape[-1]
                bv = bias_tile[0:m, n0:n0 + ncols]
                if len(sbuf.shape) == 3:
                    bv = bv.unsqueeze(1).to_broadcast(tuple(sbuf.shape))
                nc_.vector.tensor_tensor(out=sbuf, in0=sbuf, in1=bv, op=ALU.add)
            return post

        def gemm(ctx, kxm, kxn, mxn, bias_dram, nbias, transpose_kxm=True):
            bt = ctx.enter_context(tc.tile_pool(name="bias", bufs=1)).tile([P, nbias], F32)
            nc.sync.dma_start(out=bt[:].unsqueeze(1), in_=bias_dram[0:1, :].partition_broadcast(P))
            matmul_tile_kernel(
                tc, kxm[:], kxn[:], mxn[:],
                transpose_kxm=transpose_kxm, force_tensor_transpose=transpose_kxm,
                post_mxn_tile_fn=bias_adder(bt),
            )

        with ExitStack() as ctx:
            for ci in range(n_enc_cells):
                gemm(ctx, x_stage, wx_e[ci], zx_e[ci], b_e[ci], G4)
            gemm(ctx, out_emb, wx_d, g_tab, b_d, G4)

        # ---------- P2: encoder scans ----------
        with ExitStack() as ctx:
            pools = dict(
                zps=ctx.enter_context(tc.tile_pool(name="zps", bufs=3, space="PSUM")),
                tps=ctx.enter_context(tc.tile_pool(name="tps", bufs=2, space="PSUM")),
                gates=ctx.enter_context(tc.tile_pool(name="gates", bufs=3)),
                ident=ident,
            )
            zxp = ctx.enter_context(tc.tile_pool(name="zxp", bufs=4))
            ecells = [LSTMCell(nc, tc, ctx, f"e{i}", B_enc, wh_e[i]) for i in range(n_enc_cells)]
            h_fin = []
            for t in range(S):
                last = t == S - 1
                for cell, zxd_ in zip(ecells, zx_e):
                    zt = zxp.tile([B_enc, 4, 512], F32R, tag="zt")
                    nc.sync.dma_start(
                        out=zt[:],
                        in_=zxd_[t * B_enc:(t + 1) * B_enc, :].rearrange("b (n f) -> b n f", n=4))
                    h = lstm_step(nc, pools, cell, zt, skip_htranspose=last,
                                  first_step=(t == 0))
                    if last:
                        h_fin.append(h)

            # ---------- P3: handoff ----------
            if use_rs:
                nc.sync.dma_start(out=rs_h_in[:], in_=h_fin[0][:])
                nc.sync.dma_start(out=rs_c_in[:], in_=ecells[0].c[:])
                nc.gpsimd.collective_compute(
                    "ReduceScatter", ALU.add, ins=[rs_h_in[:]], outs=[rs_h_out[:]],
                    replica_groups=rs_groups)
                nc.gpsimd.collective_compute(
                    "ReduceScatter", ALU.add, ins=[rs_c_in[:]], outs=[rs_c_out[:]],
                    replica_groups=rs_groups)
                nc.sync.dma_start(out=h0[:], in_=rs_h_out[:])
                nc.sync.dma_start(out=c0[:], in_=rs_c_out[:])
            else:
                nc.vector.tensor_tensor(out=h0[:], in0=h_fin[0][:], in1=h_fin[1][:], op=ALU.add)
                nc.vector.tensor_tensor(out=c0[:], in0=ecells[0].c[:], in1=ecells[1].c[:], op=ALU.add)

        # ---------- P4: decoder scan with fused projection + softmax ----------
        with ExitStack() as ctx:
            pools = dict(
                zps=ctx.enter_context(tc.tile_pool(name="zpsd", bufs=3, space="PSUM")),
                tps=ctx.enter_context(tc.tile_pool(name="tpsd", bufs=2, space="PSUM")),
                gates=ctx.enter_context(tc.tile_pool(name="gatesd", bufs=3)),
                ident=ident,
            )
            prjps = ctx.enter_context(tc.tile_pool(name="prjps", bufs=1, space="PSUM"))
            dwp = ctx.enter_context(tc.tile_pool(name="dweights", bufs=1))
            zxp = ctx.enter_context(tc.tile_pool(name="zxpd", bufs=4))
            lstp = ctx.enter_context(tc.tile_pool(name="lstage", bufs=2))
            smp = ctx.enter_context(tc.tile_pool(name="smax", bufs=3))
            smr = ctx.enter_context(tc.tile_pool(name="smaxr", bufs=3))
            dcell = LSTMCell(nc, tc, ctx, "d", B_dec, wh_d)

            wout_sb = dwp.tile([P, 4, OUT_VOCAB], F32R, tag="wout")
            for k in range(4):
                nc.sync.dma_start(out=wout_sb[:, k, :], in_=w_out[k * P:(k + 1) * P, :])
            bto = dwp.tile([P, OUT_VOCAB], F32, tag="bto")
            nc.sync.dma_start(out=bto[:].unsqueeze(1), in_=b_out[0:1, :].partition_broadcast(P))
            yidx_sb = dwp.tile([B_dec, T], I32, tag="yidx")
            nc.sync.dma_start(out=yidx_sb[:],
                              in_=y_idx.rearrange("(t b) o -> b (t o)", b=B_dec))
            # h_T ring: 8 step slots of [128, 4(k), B]
            hstack = dwp.tile([P, 4, 8, B_dec], F32R, tag="hstack")

            transpose_into(nc, pools, h0, dcell.hT, B_dec)
            nc.vector.tensor_copy(out=dcell.c[:], in_=c0[:])

            n_grp = T // 4
            lst = None
            for t in range(T):
                zt = zxp.tile([B_dec, 4, 512], F32R, tag="zt")
                nc.gpsimd.indirect_dma_start(
                    out=zt[:].rearrange("b n f -> b (n f)"), out_offset=None,
                    in_=g_tab[:],
                    in_offset=bass.IndirectOffsetOnAxis(ap=yidx_sb[:, t:t + 1], axis=0),
                )
                hT_read = dcell.hT[:] if t == 0 else hstack[:, :, (t - 1) % 8, :]
                lstm_step(nc, pools, dcell, zt,
                          hT_read=hT_read, hT_write=hstack[:, :, t % 8, :])
                if t % 4 == 3:
                    g = t // 4
                    base = (g * 4) % 8
                    if g % (SM_GROUP // 4) == 0:
                        lst = lstp.tile([P, SM_GROUP // 4, OUT_VOCAB], F32, tag="lst")
                    lt_ps = prjps.tile([P, 2, 512], F32, tag="ppj")
                    for nn in range(2):
                        for k in range(4):
                            nc.tensor.matmul(
                                lt_ps[:, nn, :], hstack[:, k, base:base + 4, :],
                                wout_sb[:, k, nn * 512:(nn + 1) * 512],
                                start=(k == 0), stop=(k == 3))
                    gi = g % (SM_GROUP // 4)
                    nc.vector.tensor_tensor(
                        out=lst[:, gi, :].rearrange("p (s n) -> p s n", s=2),
                        in0=lt_ps[:], in1=bto[:].rearrange("p (s n) -> p s n", s=2),
                        op=ALU.add)
                if (t + 1) % SM_GROUP == 0 or t == T - 1:
                    n_m = (t % SM_GROUP) // 4 + 1
                    sg0 = (t // SM_GROUP) * SM_GROUP
                    for m in range(n_m):
                        et = smp.tile([P, OUT_VOCAB], F32, tag="et")
                        nc.scalar.activation(et[:], lst[:, m, :], AF.Exp)
                        rs = smr.tile([P, 1], F32, tag="rs")
                        nc.vector.reduce_sum(out=rs[:], in_=et[:], axis=mybir.AxisListType.X)
                        rr = smr.tile([P, 1], F32, tag="rr")
                        nc.vector.reciprocal(rr[:], rs[:])
                        ot = smp.tile([P, OUT_VOCAB], F32, tag="ot")
                        nc.vector.tensor_scalar(ot[:], et[:], rr[:], None, ALU.mult)
                        for j in range(P // B_dec):
                            nc.sync.dma_start(
                                out=out[:, sg0 + 4 * m + j, :],
                                in_=ot[j * B_dec:(j + 1) * B_dec, :],
                            )

        if debug:
            nc.sync.dma_start(out=dbg["x_stage"][:], in_=x_stage[:].bitcast(F32))
            nc.sync.dma_start(out=dbg["zx_e0"][:], in_=zx_e[0][:].bitcast(F32))
            nc.sync.dma_start(out=dbg["g_tab"][:], in_=g_tab[:].bitcast(F32))

    nc.finalize()
    return nc


_PROG_CACHE = {}


def _get_program(key):
    if key not in _PROG_CACHE:
        _PROG_CACHE[key] = build_program(*key)
    return _PROG_CACHE[key]


def kernel(inputs, teacher_actions, max_length, in_emb, out_emb,
           Wx_f, Wh_f, b_f, Wx_b, Wh_b, b_b,
           Wx_d, Wh_d, b_d, W_out, b_out, _trace=False):
    inputs = np.asarray(inputs)
    teacher_actions = np.asarray(teacher_actions)
    n_steps = S_STEPS
    assert int(max_length) == T_FULL and inputs.shape == (B_TOT, S_FULL)
    f32 = lambda a: np.ascontiguousarray(np.asarray(a), dtype=np.float32)
    in_emb, out_emb = f32(in_emb), f32(out_emb)
    Wx_f, Wh_f, Wx_b, Wh_b = f32(Wx_f), f32(Wh_f), f32(Wx_b), f32(Wh_b)
    Wx_d, Wh_d, W_out = f32(Wx_d), f32(Wh_d), f32(W_out)
    b_f, b_b, b_d, b_out = (f32(b).reshape(1, -1) for b in (b_f, b_b, b_d, b_out))

    if MODE == "quarter":
        B_enc, n_cells, B_dec, use_rs = 64, 1, 32, True
    else:
        B_enc, n_cells, B_dec, use_rs = 32, 2, 32, False
    nc = _get_program((B_enc, n_cells, B_dec, use_rs, n_steps))

    in_maps = []
    for core in range(NCORES):
        bs_d = core * B_dec
        m = dict(in_emb=in_emb, out_emb=out_emb, Wx_d=Wx_d, Wh_d=Wh_d, b_d=b_d,
                 W_out=W_out, b_out=b_out)
        m["y_idx"] = np.ascontiguousarray(
            teacher_actions[:n_steps, bs_d:bs_d + B_dec].reshape(-1, 1).astype(np.int32))
        if use_rs:
            q = core // 2
            rows = np.arange(64 * q, 64 * (q + 1))
            wx, wh, b = (Wx_f, Wh_f, b_f) if core % 2 == 0 else (Wx_b, Wh_b, b_b)
            m["Wx_e0"], m["Wh_e0"], m["b_e0"] = wx, wh, b
        else:
            rows = np.arange(bs_d, bs_d + B_enc)
            m["Wx_e0"], m["Wh_e0"], m["b_e0"] = Wx_f, Wh_f, b_f
            m["Wx_e1"], m["Wh_e1"], m["b_e1"] = Wx_b, Wh_b, b_b
        m["x_idx"] = np.ascontiguousarray(
            inputs[rows, :n_steps].T.reshape(-1, 1).astype(np.int32))
        in_maps.append(m)

    res = run_bass_kernel_spmd(nc, in_maps, core_ids=list(range(NCORES)), trace=_trace)
    out = np.concatenate([res.results[i]["out"] for i in range(NCORES)], axis=0)
    if _trace:
        kernel.last_exec_time_ns = res.exec_time_ns
        kernel.last_results = res
    return out
